# revision 31
# baseline (speedup 1.0000x reference)
"""GAT 2-layer kernel for Trainium2, 8 NeuronCores — v3.

Edge gathers use the gpsimd dma_gather ucode (int16 indices wrapped in 16
partitions and replicated across the 8 Q7 cores, 256B table rows, 1024
indices per instruction, rotated over 4 SWDGE queues).  Node indices beyond
int16 range are handled by splitting every tile's edge rows into a lo region
(src < 32768, gathered from T[0:32768]) and a hi region (gathered from
T[32768:]).  Padded slots gather row 0 and are zeroed by a mask multiply.
a_dst is produced on-device: per-tile a_dst vectors are fetched with tiny
per-column indirect DMAs, then expanded to edge rows by one-hot (S^T)
matmuls on the tensor engine.
"""

import numpy as np
import ml_dtypes
from contextlib import ExitStack

import concourse.bass as bass
import concourse.tile as tile
from concourse import bacc, mybir
from concourse.bass import ts, ds
from concourse.bass_utils import run_bass_kernel_spmd

BF16 = mybir.dt.bfloat16
F32 = mybir.dt.float32
I32 = mybir.dt.int32
I16 = mybir.dt.int16
NPBF16 = ml_dtypes.bfloat16

P = 128
NCORES = 8
N = 50000
E = 1600000
TPC = 49
NPAD = NCORES * TPC * P       # 50176
L = 4                         # edge slots per row
GSZ = 2                       # dst tiles per edge-phase group
BT = 4                        # node tiles per build iteration
NIDX_MAX = 1024               # dma_gather instruction size cap
HYB_COLS = 0                  # hi-region columns per group routed via
                              # per-column indirect DMA (disabled: SWDGE
                              # semaphore lanes can't mix queues in flight)
SPLIT = 32768                 # lo/hi region boundary (int16 index range)
NEG_SLOPE = 0.2
TW = 128                      # table row width (256B)

LAST_RESULTS = []


def _prep_edges(edge_index, n_nodes=N, npad=NPAD, tpc=TPC, ncores=NCORES):
    src = np.asarray(edge_index[0]).astype(np.int64)
    dst = np.asarray(edge_index[1]).astype(np.int64)
    order = np.argsort(dst, kind="stable")
    srcs = src[order].astype(np.int64)
    dsts = dst[order]
    g_tiles = ncores * tpc
    split = min(SPLIT, npad)

    # secondary sort: within each dst, lo-src edges first
    is_hi = (srcs >= split).astype(np.int64)
    order2 = np.lexsort((is_hi, dsts))
    srcs = srcs[order2]
    dsts = dsts[order2]
    is_hi = is_hi[order2]

    deg_lo = np.bincount(dsts[is_hi == 0], minlength=npad)
    deg_hi = np.bincount(dsts[is_hi == 1], minlength=npad)
    rows_lo_n = (deg_lo + L - 1) // L
    rows_hi_n = (deg_hi + L - 1) // L

    def tile_chunks(rows_n):
        gro = np.concatenate([[0], np.cumsum(rows_n)])
        t0 = gro[np.arange(g_tiles) * P]
        rt = gro[np.minimum(np.arange(1, g_tiles + 1) * P, npad)] - t0
        ch = (rt + P - 1) // P
        return gro, t0, ch.reshape(ncores, tpc).max(axis=0)

    gro_lo, tstart_lo, RchL = tile_chunks(rows_lo_n)
    gro_hi, tstart_hi, RchH = tile_chunks(rows_hi_n)
    RchL = np.maximum(RchL, 1)          # keep every tile non-empty

    # ---- global chunk order: per group, lo block then hi block ----
    groups = []
    t0 = 0
    while t0 < tpc:
        groups.append((t0, min(GSZ, tpc - t0)))
        t0 += GSZ
    chunk_base = np.zeros((tpc, 2), np.int64)   # (t, region) -> first cid
    tile_chunk_lists = [[] for _ in range(tpc)]
    groups_meta = []
    cid = 0
    hicum = 0
    for (t0, gn) in groups:
        gbase = cid
        specs = []
        for ri, Rch in ((0, RchL), (1, RchH)):
            col0 = (cid - gbase) * L
            ncols = int(sum(Rch[t0 + j] for j in range(gn))) * L
            if ncols:
                specs.append((ri, col0, ncols, cid - gbase,
                              hicum if ri == 1 else 0))
                if ri == 1:
                    hicum += ncols
            for j in range(gn):
                chunk_base[t0 + j, ri] = cid
                for rr in range(int(Rch[t0 + j])):
                    tile_chunk_lists[t0 + j].append(cid)
                    cid += 1
        groups_meta.append((t0, gn, gbase, cid - gbase, specs))
    NRCH = cid
    NHICOL = hicum

    rdf_arr = np.full((ncores, P, NRCH), -1.0, np.float32)
    rdfR_arr = np.full((ncores, NRCH * P), -1.0, np.float32)
    mask_arr = np.zeros((ncores, P, NRCH * L), np.float32)
    idxs_arr = np.zeros((ncores, NRCH * L * P), np.int64)

    for ri, rows_n, gro, tstart in (
            (0, rows_lo_n, gro_lo, tstart_lo),
            (1, rows_hi_n, gro_hi, tstart_hi)):
        tot = int(gro[-1])
        if tot == 0:
            continue
        row_node = np.repeat(np.arange(npad), rows_n)
        row_tile = row_node >> 7
        rit = np.arange(tot) - tstart[row_tile]
        r_core = row_tile // tpc
        r_t = row_tile % tpc
        r_cid = chunk_base[r_t, ri] + rit // P
        r_p = rit % P
        rdf_arr[r_core, r_p, r_cid] = (row_node & 127).astype(np.float32)
        rdfR_arr[r_core, r_cid * P + r_p] = (row_node & 127).astype(np.float32)

    # per-edge slot fill
    node_start_lo = np.concatenate([[0], np.cumsum(deg_lo)])
    node_start_hi = np.concatenate([[0], np.cumsum(deg_hi)])
    ne = len(srcs)
    e_t = dsts >> 7
    e_core = e_t // tpc
    e_tl = e_t % tpc
    for ri, sel, gro, tstart, node_start in (
            (0, is_hi == 0, gro_lo, tstart_lo, node_start_lo),
            (1, is_hi == 1, gro_hi, tstart_hi, node_start_hi)):
        idx_e = np.where(sel)[0]
        if len(idx_e) == 0:
            continue
        d = dsts[idx_e]
        s = srcs[idx_e] - (0 if ri == 0 else split)
        # rank within (node, region): edges of one node in this region are
        # consecutive after the lexsort
        pos = np.arange(len(idx_e))
        seg0 = np.searchsorted(d, np.arange(npad), side="left")
        rank = pos - seg0[d]
        row_r = gro[d] + rank // L
        slot = rank % L
        rit = row_r - tstart[e_t[idx_e]]
        cidv = chunk_base[e_tl[idx_e], ri] + rit // P
        pv = rit % P
        mask_arr[e_core[idx_e], pv, cidv * L + slot] = 1.0
        idxs_arr[e_core[idx_e], (cidv * L + slot) * P + pv] = s

    # int16 wrap + 8x replication
    TOT = NRCH * L * P
    assert TOT % 16 == 0
    idx16_arr = np.zeros((ncores, P, TOT // 16), np.int16)
    for k in range(ncores):
        w = idxs_arr[k].astype(np.int16).reshape(TOT // 16, 16).T
        for rep in range(8):
            idx16_arr[k, 16 * rep:16 * rep + 16, :] = w

    adsti = (np.arange(ncores)[:, None, None] * tpc * P
             + np.arange(tpc)[None, None, :] * P
             + np.arange(P)[None, :, None]).astype(np.int32)

    # global (i32) indices for the indirect-routed hi columns
    hoff_arr = np.zeros((ncores, P, max(NHICOL, 1)), np.int32)
    for (t0, gn, gbase, RL, specs) in groups_meta:
        for (ri, col0, ncols, chunk0, hbase) in specs:
            if ri != 1:
                continue
            g0 = (gbase + chunk0) * L * P
            blk = idxs_arr[:, g0:g0 + ncols * P].reshape(ncores, ncols, P)
            hoff_arr[:, :, hbase:hbase + ncols] = \
                blk.transpose(0, 2, 1).astype(np.int32) + split

    return dict(
        rdf=rdf_arr.astype(NPBF16), rdfR=rdfR_arr.astype(NPBF16),
        mask=mask_arr.astype(NPBF16), idx16=idx16_arr, adsti=adsti,
        hoff=hoff_arr, NHICOL=NHICOL,
        NRCH=NRCH, groups_meta=groups_meta,
        tile_chunk_lists=tile_chunk_lists,
        RLG_MAX=max(g[3] for g in groups_meta),
        RCHT_MAX=max(len(c) for c in tile_chunk_lists),
    )


def _build_layer_program(KIN, F_D, meta, layer, npad=NPAD, tpc=TPC,
                         ncores=NCORES):
    F_H = 64
    F_G = F_H + F_D          # aggregated row width (msg | w)
    MW = L * F_G             # per-tile psum width
    KT = (KIN + P - 1) // P
    KP = min(KIN, P)
    NRCH = meta["NRCH"]
    RLG_MAX = meta["RLG_MAX"]
    RCHT_MAX = meta["RCHT_MAX"]
    g_tiles = ncores * tpc
    n_bt = (g_tiles + BT - 1) // BT
    split = min(SPLIT, npad)
    out_dt = BF16 if layer == 1 else F32
    eps = 1e-16

    nc = bacc.Bacc("TRN2", target_bir_lowering=False, debug=False,
                   num_devices=ncores, num_swdge_queues=4,
                   dynamic_dma_scratch_size=32768)

    xT_in = nc.dram_tensor("xT", [KIN, npad], BF16, kind="ExternalInput").ap()
    wc_in = nc.dram_tensor("wc", [KIN, TW], BF16, kind="ExternalInput").ap()
    idx_in = nc.dram_tensor("idx16", [P, NRCH * L * P // 16], I16,
                            kind="ExternalInput").ap()
    rdf_in = nc.dram_tensor("rdf", [P, NRCH], BF16, kind="ExternalInput").ap()
    rdfR_in = nc.dram_tensor("rdfR", [1, NRCH * P], BF16,
                             kind="ExternalInput").ap()
    mask_in = nc.dram_tensor("mask", [P, NRCH * L], BF16,
                             kind="ExternalInput").ap()
    adsti_in = nc.dram_tensor("adsti", [P, tpc], I32,
                              kind="ExternalInput").ap()
    NHICOL = meta["NHICOL"]
    hoff_in = nc.dram_tensor("hoff", [P, max(NHICOL, 1)], I32,
                             kind="ExternalInput").ap()
    bias_in = nc.dram_tensor("bias", [1, F_H], F32, kind="ExternalInput").ap()
    out_dram = nc.dram_tensor("out", [tpc * P, F_H], out_dt,
                              kind="ExternalOutput").ap()

    with tile.TileContext(nc) as tc, ExitStack() as ctx:
        cpool = ctx.enter_context(tc.tile_pool(name="const", bufs=1))
        dpool = ctx.enter_context(tc.tile_pool(name="dram", bufs=1,
                                               space=bass.MemorySpace.DRAM))
        bpool = ctx.enter_context(tc.tile_pool(name="bld", bufs=3))
        epool = ctx.enter_context(tc.tile_pool(name="edge", bufs=2))
        opool = ctx.enter_context(tc.tile_pool(name="post", bufs=2))
        pps = ctx.enter_context(tc.tile_pool(name="psb", bufs=2,
                                             space=bass.MemorySpace.PSUM))
        ppe = ctx.enter_context(tc.tile_pool(name="pse", bufs=2,
                                             space=bass.MemorySpace.PSUM))
        pat = ctx.enter_context(tc.tile_pool(name="pat", bufs=2,
                                             space=bass.MemorySpace.PSUM))

        # ---- constants ----
        wc_sb = cpool.tile([KP, KT, TW], BF16)
        for kt in range(KT):
            nc.sync.dma_start(wc_sb[:, kt, :], wc_in[kt * KP:(kt + 1) * KP, :])
        bias_sb = cpool.tile([P, F_H], F32)
        nc.sync.dma_start(bias_sb[:], bias_in.to_broadcast((P, F_H)))
        iota_i = cpool.tile([P, 128], I32)
        nc.gpsimd.iota(iota_i[:], pattern=[[1, 128]], channel_multiplier=0)
        iota_bf = cpool.tile([P, 1, 128], BF16)
        nc.vector.tensor_copy(iota_bf[:, 0, :], iota_i[:])
        iop_i = cpool.tile([P, 1], I32)
        nc.gpsimd.iota(iop_i[:], pattern=[[0, 1]], channel_multiplier=1)
        iop_bf = cpool.tile([P, 1, 1], BF16)
        nc.vector.tensor_copy(iop_bf[:, 0, :], iop_i[:])
        idx16_sb = cpool.tile([P, NRCH * L * P // 16], I16)
        nc.sync.dma_start(idx16_sb[:], idx_in[:])
        mask_sb = cpool.tile([P, NRCH * L], BF16)
        nc.sync.dma_start(mask_sb[:], mask_in[:])
        rdf_sb = cpool.tile([P, NRCH, 1], BF16)
        nc.sync.dma_start(rdf_sb[:, :, 0], rdf_in[:])
        adsti_sb = cpool.tile([P, tpc], I32)
        nc.sync.dma_start(adsti_sb[:], adsti_in[:])
        hoff_sb = cpool.tile([P, max(NHICOL, 1)], I32)
        nc.sync.dma_start(hoff_sb[:], hoff_in[:])

        # ---- phase 1: build T = [h | a_src | a_dst | 0pad] ----
        T_dram = dpool.tile([npad, TW], BF16)
        for b in range(n_bt):
            bt = min(BT, g_tiles - b * BT)
            cols = bt * P
            xt = bpool.tile([KP, KT, BT, P], BF16)
            for kt in range(KT):
                nc.sync.dma_start(
                    xt[:, kt, 0:bt, :],
                    xT_in[kt * KP:(kt + 1) * KP, ds(b * BT * P, cols)])
            tcast = bpool.tile([P, BT, TW], BF16)
            for j in range(bt):
                psB = pps.tile([P, TW], F32)
                for kt in range(KT):
                    nc.tensor.matmul(psB[:], xt[:, kt, j, :], wc_sb[:, kt, :],
                                     start=(kt == 0), stop=(kt == KT - 1))
                if j % 2 == 0:
                    nc.vector.tensor_copy(tcast[:, j, :], psB[:])
                else:
                    nc.scalar.activation(tcast[:, j, :], psB[:],
                                         mybir.ActivationFunctionType.Copy)
            nc.scalar.dma_start(
                T_dram[ds(b * BT * P, cols), :].rearrange(
                    "(j p) c -> p j c", p=P),
                tcast[:, 0:bt, :])

        # ---- per-tile a_dst vectors (rows are contiguous; tiny gathers) ----
        A_all = cpool.tile([P, tpc, TW], BF16)
        for t in range(tpc):
            nc.gpsimd.indirect_dma_start(
                out=A_all[:, t, :], out_offset=None, in_=T_dram[:],
                in_offset=bass.IndirectOffsetOnAxis(
                    ap=adsti_sb[:, t:t + 1], axis=0))

        # ---- phase 2: edge aggregation per group ----
        if layer == 2:
            zbuf = cpool.tile([P, tpc, F_H], F32)
            sums = cpool.tile([P, tpc], F32)

        CW = F_H // F_D
        tile_chunks = meta["tile_chunk_lists"]
        qrot = [0]
        for (t0, gn, gbase, RL, specs) in meta["groups_meta"]:
            RLs = RL * L
            Gt = epool.tile([P, RLG_MAX * L, TW], BF16)
            for (ri, col0, ncols, chunk0, hbase) in specs:
                # route the tail of the hi region through per-column indirect
                # DMAs so its packets ride the 12 DMA engines the 4 dma_gather
                # queues don't use
                nhyb = min(HYB_COLS, ncols) if ri == 1 else 0
                nslots = (ncols - nhyb) * P
                islot0 = (gbase + chunk0) * L * P
                tab = T_dram[0:split, :] if ri == 0 \
                    else T_dram[split:npad, :]
                for c in range(ncols - nhyb, ncols):
                    nc.gpsimd.indirect_dma_start(
                        out=Gt[:, col0 + c, :], out_offset=None,
                        in_=T_dram[:],
                        in_offset=bass.IndirectOffsetOnAxis(
                            ap=hoff_sb[:, hbase + c:hbase + c + 1], axis=0))
                o = 0
                while o < nslots:
                    ni = min(NIDX_MAX, nslots - o)
                    i0 = (islot0 + o) // 16
                    nc.gpsimd.dma_gather(
                        out_ap=Gt[:, col0 + o // P:col0 + (o + ni) // P, :],
                        in_ap=tab,
                        idxs_ap=idx16_sb[:, i0:i0 + ni // 16],
                        num_idxs=ni, num_idxs_reg=ni, elem_size=TW,
                        queue_num=qrot[0] % 4)
                    qrot[0] += 1
                    o += ni

            # S^T for a_dst expansion: S_T[d, r] = (rowdst[r] == d)
            rdfR_sb = epool.tile([P, RLG_MAX, 128], BF16)
            nc.scalar.dma_start(
                rdfR_sb[:, 0:RL, :].rearrange("p r d -> p (r d)"),
                rdfR_in[0:1, ds(gbase * P, RL * P)].to_broadcast(
                    (P, RL * P)))
            S_T = epool.tile([P, RLG_MAX, 128], BF16)
            nc.vector.tensor_tensor(
                S_T[:, 0:RL, :],
                rdfR_sb[:, 0:RL, :],
                iop_bf[:].to_broadcast((P, RL, 128)),
                op=mybir.AluOpType.is_equal)

            atsb = epool.tile([P, RLG_MAX, F_D], F32)
            for j in range(gn):
                t = t0 + j
                chunks = tile_chunks[t]
                psAt = pat.tile([P, RCHT_MAX * F_D], F32)
                for ci, cid in enumerate(chunks):
                    nc.tensor.matmul(
                        psAt[:, ci * F_D:(ci + 1) * F_D],
                        S_T[:, cid - gbase, :],
                        A_all[:, t, F_H + F_D:F_H + 2 * F_D],
                        start=True, stop=True)
                runs, st = [], chunks[0]
                prev = st
                for c in chunks[1:]:
                    if c == prev + 1:
                        prev = c
                        continue
                    runs.append((st, prev))
                    st = prev = c
                runs.append((st, prev))
                for (a, b) in runs:
                    nch = b - a + 1
                    ci0 = chunks.index(a)
                    nc.vector.tensor_copy(
                        atsb[:, a - gbase:a - gbase + nch, :],
                        psAt[:, ci0 * F_D:(ci0 + nch) * F_D].rearrange(
                            "p (c f) -> p c f", f=F_D))

            # logits -> w = exp(leakyrelu(a_src + a_dst)) * mask
            Lt = epool.tile([P, RLG_MAX * L, F_D], BF16)
            nc.vector.tensor_add(
                Lt[:, 0:RLs, :].rearrange("p (r l) f -> p r l f", l=L),
                Gt[:, 0:RLs, F_H:F_H + F_D].rearrange(
                    "p (r l) f -> p r l f", l=L),
                atsb[:, 0:RL, :].unsqueeze(2).to_broadcast((P, RL, L, F_D)))
            L2t = epool.tile([P, RLG_MAX * L, F_D], BF16)
            nc.vector.tensor_scalar_mul(L2t[:, 0:RLs, :], Lt[:, 0:RLs, :],
                                        NEG_SLOPE)
            nc.vector.tensor_tensor(L2t[:, 0:RLs, :], Lt[:, 0:RLs, :],
                                    L2t[:, 0:RLs, :], op=mybir.AluOpType.max)
            nc.scalar.activation(Gt[:, 0:RLs, F_H:F_H + F_D],
                                 L2t[:, 0:RLs, :],
                                 mybir.ActivationFunctionType.Exp)
            nc.vector.tensor_mul(
                Gt[:, 0:RLs, F_H:F_H + F_D],
                Gt[:, 0:RLs, F_H:F_H + F_D],
                mask_sb[:, ds(gbase * L, RLs)].unsqueeze(2).to_broadcast(
                    (P, RLs, F_D)))
            # msg = w * h[src]  (keep off gpsimd — it is the gather engine)
            for h in range(F_D):
                eng = nc.vector
                eng.tensor_mul(
                    Gt[:, 0:RLs, h * CW:(h + 1) * CW],
                    Gt[:, 0:RLs, h * CW:(h + 1) * CW],
                    Gt[:, 0:RLs, F_H + h:F_H + h + 1].to_broadcast(
                        (P, RLs, CW)))
            # aggregation selection matrix S[r, d]
            St = epool.tile([P, RLG_MAX, 128], BF16)
            nc.vector.tensor_tensor(
                St[:, 0:RL, :],
                rdf_sb[:, ds(gbase, RL), :].to_broadcast((P, RL, 128)),
                iota_bf[:].to_broadcast((P, RL, 128)),
                op=mybir.AluOpType.is_equal)

            red = opool.tile([P, GSZ, F_G, 1], F32)
            for j in range(gn):
                t = t0 + j
                chunks = tile_chunks[t]
                psE = ppe.tile([P, MW], F32)
                for ci, cid in enumerate(chunks):
                    rc = cid - gbase
                    nc.tensor.matmul(
                        psE[:], St[:, rc, :],
                        Gt[:, rc * L:(rc + 1) * L, 0:F_G],
                        start=(ci == 0), stop=(ci == len(chunks) - 1))
                nc.vector.tensor_reduce(
                    red[:, j, :, :],
                    psE[:].rearrange("p (l c) -> p c l", l=L),
                    mybir.AxisListType.X, mybir.AluOpType.add)

            # ---- postprocess ----
            den = opool.tile([P, GSZ, F_D], F32)
            nc.vector.tensor_scalar_add(den[:, 0:gn, :],
                                        red[:, 0:gn, F_H:F_G, 0], eps)
            rec = opool.tile([P, GSZ, F_D], F32)
            nc.vector.reciprocal(rec[:, 0:gn, :], den[:, 0:gn, :])
            o1 = opool.tile([P, GSZ, F_H], F32)
            for h in range(F_D):
                nc.vector.tensor_mul(
                    o1[:, 0:gn, h * CW:(h + 1) * CW],
                    red[:, 0:gn, h * CW:(h + 1) * CW, 0],
                    rec[:, 0:gn, h:h + 1].to_broadcast((P, gn, CW)))
            nc.vector.tensor_add(o1[:, 0:gn, :], o1[:, 0:gn, :],
                                 bias_sb[:].unsqueeze(1).to_broadcast(
                                     (P, gn, F_H)))
            if layer == 1:
                mn = opool.tile([P, GSZ, F_H], F32)
                nc.vector.tensor_scalar_min(mn[:, 0:gn, :], o1[:, 0:gn, :],
                                            0.0)
                em = opool.tile([P, GSZ, F_H], F32)
                nc.scalar.activation(em[:, 0:gn, :], mn[:, 0:gn, :],
                                     mybir.ActivationFunctionType.Exp)
                mx = opool.tile([P, GSZ, F_H], F32)
                nc.vector.tensor_scalar_max(mx[:, 0:gn, :], o1[:, 0:gn, :],
                                            0.0)
                s1 = opool.tile([P, GSZ, F_H], F32)
                nc.vector.tensor_add(s1[:, 0:gn, :], mx[:, 0:gn, :],
                                     em[:, 0:gn, :])
                ob = opool.tile([P, GSZ, F_H], BF16)
                nc.vector.tensor_scalar_add(ob[:, 0:gn, :], s1[:, 0:gn, :],
                                            -1.0)
                nc.scalar.dma_start(
                    out_dram[ds(t0 * P, gn * P), :].rearrange(
                        "(j p) c -> p j c", p=P),
                    ob[:, 0:gn, :])
            else:
                rm = opool.tile([P, GSZ, 1], F32)
                nc.vector.tensor_reduce(rm[:, 0:gn, :], o1[:, 0:gn, :],
                                        mybir.AxisListType.X,
                                        mybir.AluOpType.max)
                nc.vector.tensor_tensor(
                    zbuf[:, t0:t0 + gn, :], o1[:, 0:gn, :],
                    rm[:, 0:gn, :].to_broadcast((P, gn, F_H)),
                    op=mybir.AluOpType.subtract)
                for j in range(gn):
                    nc.scalar.activation(
                        o1[:, j, :], zbuf[:, t0 + j, :],
                        mybir.ActivationFunctionType.Exp,
                        accum_out=sums[:, t0 + j:t0 + j + 1])

        if layer == 2:
            ls = cpool.tile([P, tpc], F32)
            nc.scalar.activation(ls[:], sums[:],
                                 mybir.ActivationFunctionType.Ln)
            for (t0, gn, gbase, RL, specs) in meta["groups_meta"]:
                of = opool.tile([P, GSZ, F_H], F32)
                nc.vector.tensor_tensor(
                    of[:, 0:gn, :], zbuf[:, t0:t0 + gn, :],
                    ls[:, t0:t0 + gn].unsqueeze(2).to_broadcast(
                        (P, gn, F_H)),
                    op=mybir.AluOpType.subtract)
                nc.scalar.dma_start(
                    out_dram[ds(t0 * P, gn * P), :].rearrange(
                        "(j p) c -> p j c", p=P),
                    of[:, 0:gn, :])

    nc.compile()
    return nc


def _fold_weights1(W1, att_src1, att_dst1):
    A1s = np.zeros((64, 8), np.float32)
    A1s[np.arange(64), np.arange(64) // 8] = att_src1.reshape(64)
    A1d = np.zeros((64, 8), np.float32)
    A1d[np.arange(64), np.arange(64) // 8] = att_dst1.reshape(64)
    wc = np.zeros((256, TW), np.float32)
    wc[:, 0:64] = W1
    wc[:, 64:72] = W1 @ A1s
    wc[:, 72:80] = W1 @ A1d
    return wc


def kernel(x, edge_index, W1, att_src1, att_dst1, bias1,
           W2, att_src2, att_dst2, bias2):
    LAST_RESULTS.clear()
    meta = _prep_edges(edge_index)

    def in_maps_for(xT, wc, bias):
        return [{
            "xT": xT, "wc": wc,
            "idx16": np.ascontiguousarray(meta["idx16"][k]),
            "rdf": np.ascontiguousarray(meta["rdf"][k]),
            "rdfR": np.ascontiguousarray(meta["rdfR"][k]).reshape(1, -1),
            "mask": np.ascontiguousarray(meta["mask"][k]),
            "adsti": np.ascontiguousarray(meta["adsti"][k]),
            "hoff": np.ascontiguousarray(meta["hoff"][k]),
            "bias": np.asarray(bias, np.float32).reshape(1, 64),
        } for k in range(NCORES)]

    # ---------- layer 1 ----------
    Wc1 = _fold_weights1(W1, att_src1, att_dst1).astype(NPBF16)
    xT = np.zeros((256, NPAD), NPBF16)
    xT[:, :N] = np.asarray(x).T.astype(NPBF16)
    nc1 = _build_layer_program(256, 8, meta, layer=1)
    res1 = run_bass_kernel_spmd(nc1, in_maps_for(xT, Wc1, bias1),
                                core_ids=list(range(NCORES)))
    LAST_RESULTS.append(res1)
    x2 = np.concatenate([res1.results[k]["out"] for k in range(NCORES)],
                        axis=0)

    # ---------- layer 2 ----------
    Wc2 = np.zeros((64, TW), np.float32)
    Wc2[:, 0:64] = W2
    Wc2[:, 64:65] = W2 @ att_src2.T
    Wc2[:, 65:66] = W2 @ att_dst2.T
    Wc2 = Wc2.astype(NPBF16)
    x2T = np.ascontiguousarray(x2.T)
    nc2 = _build_layer_program(64, 1, meta, layer=2)
    res2 = run_bass_kernel_spmd(nc2, in_maps_for(x2T, Wc2, bias2),
                                core_ids=list(range(NCORES)))
    LAST_RESULTS.append(res2)
    out = np.concatenate([res2.results[k]["out"] for k in range(NCORES)],
                         axis=0)
    return out[:N].astype(np.float32)
